# revision 17
# baseline (speedup 1.0000x reference)
"""Trainium2 Bass kernel for nn_BERTStudentPruner.

Computation (per the reference):
    h1 = relu(x @ W1.T + b1)            x: [B, S, D]
    h2 = relu(h1 @ W2.T + b2)
    y_soft = sigmoid(h2 @ W3.T + b3)    -> [B, S]
    y_hard = rank(y_soft, per-row stable ascending) < k, with token 0 of each
             row forced below the min so it is always kept.

Strategy: pure data parallelism over the batch dim (8 batches per core on 8
NeuronCores).  Per core, a fully-fused fp32 MLP (PE transposes the activation
tiles; weights are host-pre-transposed) followed by an exact on-device top-k
mask built from a 32-step bitwise binary search over the monotone int32
mapping of the logits (ties broken by index via a prefix-sum, exactly
matching stable argsort semantics).
"""

import sys

for _p in ("/opt/trn_rl_repo",):
    if _p not in sys.path:
        sys.path.insert(0, _p)

import numpy as np

B, S, D = 64, 2048, 768
N_CORES = 8
B_LOCAL = B // N_CORES


def _build_nc(B_local, S, D, k, has_b1, has_b2, b3val):
    from contextlib import ExitStack

    import concourse.bass as bass
    import concourse.bacc as bacc
    import concourse.mybir as mybir
    import concourse.tile as tile

    f32 = mybir.dt.float32
    i32 = mybir.dt.int32
    u8 = mybir.dt.uint8
    Alu = mybir.AluOpType
    Act = mybir.ActivationFunctionType
    AxX = mybir.AxisListType.X

    T = B_local * S          # tokens per core
    KC = D // 128            # feature chunks
    NG = T // 512            # 512-token groups
    NP = T // 128            # partitions used in the [NP, 128] token layout
    kp = k - 1               # slots left after the always-kept token 0
    assert T % 512 == 0 and D % 128 == 0 and NP <= 128 and S % 128 == 0

    nc = bacc.Bacc()
    x_d = nc.declare_dram_parameter("x", [T, D], f32, isOutput=False)
    w1_d = nc.declare_dram_parameter("w1t", [D, D], f32, isOutput=False)
    w2_d = nc.declare_dram_parameter("w2t", [D, D], f32, isOutput=False)
    w3_d = nc.declare_dram_parameter("w3e", [128, KC * 16], f32, isOutput=False)
    id_d = nc.declare_dram_parameter("ident", [128, 128], f32, isOutput=False)
    hm_d = nc.declare_dram_parameter("hmat", [128, 128], f32, isOutput=False)
    tx_d = nc.declare_dram_parameter("texcl", [128, 128], f32, isOutput=False)
    fb_d = nc.declare_dram_parameter("fbig", [128, 128], f32, isOutput=False)
    fo_d = nc.declare_dram_parameter("fone", [128, 128], f32, isOutput=False)
    ci_d = nc.declare_dram_parameter("ci", [128, 4], i32, isOutput=False)
    if has_b1:
        b1_d = nc.declare_dram_parameter("b1r", [1, D], f32, isOutput=False)
    if has_b2:
        b2_d = nc.declare_dram_parameter("b2r", [1, D], f32, isOutput=False)
    if has_b1 or has_b2:
        on_d = nc.declare_dram_parameter("ones", [1, 512], f32, isOutput=False)
    ys_d = nc.declare_dram_parameter("y_soft", [T], f32, isOutput=True)
    yh_d = nc.declare_dram_parameter("y_hard", [T], u8, isOutput=True)

    with ExitStack() as ctx:
        tc = ctx.enter_context(tile.TileContext(nc))
        cpool = ctx.enter_context(tc.tile_pool(name="consts", bufs=1))

        w1sb = cpool.tile([128, KC, D], f32)
        nc.sync.dma_start(out=w1sb, in_=w1_d[:].rearrange("(kc p) e -> p kc e", p=128))
        w2sb = cpool.tile([128, KC, D], f32)
        nc.sync.dma_start(out=w2sb, in_=w2_d[:].rearrange("(kc p) e -> p kc e", p=128))
        w3sb = cpool.tile([128, KC, 4, 4], f32)
        nc.sync.dma_start(out=w3sb, in_=w3_d[:].rearrange("p (kc s m) -> p kc s m", s=4, m=4))
        ident = cpool.tile([128, 128], f32)
        nc.sync.dma_start(out=ident, in_=id_d[:])
        hmat = cpool.tile([128, 128], f32)
        nc.sync.dma_start(out=hmat, in_=hm_d[:])
        texcl = cpool.tile([128, 128], f32)
        nc.sync.dma_start(out=texcl, in_=tx_d[:])
        fbig = cpool.tile([128, 128], f32)
        nc.sync.dma_start(out=fbig, in_=fb_d[:])
        fone = cpool.tile([128, 128], f32)
        nc.sync.dma_start(out=fone, in_=fo_d[:])
        ci = cpool.tile([128, 4], i32)
        nc.sync.dma_start(out=ci, in_=ci_d[:])
        if has_b1:
            b1sb = cpool.tile([1, D], f32)
            nc.sync.dma_start(out=b1sb, in_=b1_d[:])
        if has_b2:
            b2sb = cpool.tile([1, D], f32)
            nc.sync.dma_start(out=b2sb, in_=b2_d[:])
        if has_b1 or has_b2:
            onesb = cpool.tile([1, 512], f32)
            nc.sync.dma_start(out=onesb, in_=on_d[:])

        z2 = cpool.tile([NP, 128], f32)

        xpool = ctx.enter_context(tc.tile_pool(name="xn", bufs=2))
        xtpool = ctx.enter_context(tc.tile_pool(name="xt", bufs=2))
        h1pool = ctx.enter_context(tc.tile_pool(name="h1", bufs=2))
        h2pool = ctx.enter_context(tc.tile_pool(name="h2", bufs=2))
        psT = ctx.enter_context(tc.tile_pool(name="psT", bufs=2, space="PSUM"))
        psMM = ctx.enter_context(tc.tile_pool(name="psMM", bufs=3, space="PSUM"))
        psZ = ctx.enter_context(tc.tile_pool(name="psZ", bufs=2, space="PSUM"))
        zrpool = ctx.enter_context(tc.tile_pool(name="zr", bufs=2))

        for g in range(NG):
            xn = xpool.tile([128, 4, D], f32, tag="xn", name=f"xn{g}")
            nc.sync.dma_start(
                out=xn,
                in_=x_d[g * 512 : (g + 1) * 512, :].rearrange("(s p) e -> p s e", p=128),
            )
            # transpose x tiles: [token, d] -> [d, token]
            xts = []
            for kc in range(KC):
                xt = xtpool.tile([128, 512], f32, tag=f"xt{kc}", name=f"xt{g}_{kc}")
                ptg = psT.tile([128, 512], f32, tag="pt", name=f"pt{g}_{kc}")
                for s in range(4):
                    nc.tensor.transpose(
                        ptg[:, s * 128 : (s + 1) * 128],
                        xn[:, s, kc * 128 : (kc + 1) * 128],
                        ident,
                    )
                nc.vector.tensor_copy(xt, ptg)
                xts.append(xt)
            # layer 1
            h1s = []
            for ec in range(KC):
                ps = psMM.tile([128, 512], f32, tag="mm", name=f"p1_{g}_{ec}")
                for kc in range(KC):
                    nc.tensor.matmul(
                        ps,
                        w1sb[:, kc, ec * 128 : (ec + 1) * 128],
                        xts[kc],
                        start=(kc == 0),
                        stop=(kc == KC - 1 and not has_b1),
                    )
                if has_b1:
                    nc.tensor.matmul(
                        ps, b1sb[:, ec * 128 : (ec + 1) * 128], onesb,
                        start=False, stop=True,
                    )
                h1 = h1pool.tile([128, 512], f32, tag=f"h1{ec}", name=f"h1_{g}_{ec}")
                nc.scalar.activation(h1, ps, Act.Relu)
                h1s.append(h1)
            # layer 2
            h2s = []
            for fc in range(KC):
                ps = psMM.tile([128, 512], f32, tag="mm", name=f"p2_{g}_{fc}")
                for kc in range(KC):
                    nc.tensor.matmul(
                        ps,
                        w2sb[:, kc, fc * 128 : (fc + 1) * 128],
                        h1s[kc],
                        start=(kc == 0),
                        stop=(kc == KC - 1 and not has_b2),
                    )
                if has_b2:
                    nc.tensor.matmul(
                        ps, b2sb[:, fc * 128 : (fc + 1) * 128], onesb,
                        start=False, stop=True,
                    )
                h2 = h2pool.tile([128, 512], f32, tag=f"h2{fc}", name=f"h2_{g}_{fc}")
                nc.scalar.activation(h2, ps, Act.Relu)
                h2s.append(h2)
            # layer 3: z[t] = w3 . h2[t]; routed so subtile s lands on psum
            # partition s (lhsT column s holds w3, other columns zero)
            z4 = psZ.tile([4, 128], f32, tag="z4", name=f"z4_{g}")
            first = True
            for kc in range(KC):
                for s in range(4):
                    nc.tensor.matmul(
                        z4,
                        w3sb[:, kc, s, :],
                        h2s[kc][:, s * 128 : (s + 1) * 128],
                        start=first,
                        stop=(kc == KC - 1 and s == 3),
                    )
                    first = False
            zrow = zrpool.tile([4, 128], f32, tag="zr", name=f"zr{g}")
            nc.scalar.activation(zrow, z4, Act.Copy, bias=float(b3val))
            nc.sync.dma_start(out=z2[4 * g : 4 * g + 4, :], in_=zrow)

        # ---- top-k mask on z2 [NP, 128]; token t = 128*p + f ----
        kpool = ctx.enter_context(tc.tile_pool(name="kp", bufs=1))
        psK = ctx.enter_context(tc.tile_pool(name="psK", bufs=1, space="PSUM"))

        z2m = kpool.tile([NP, 128], f32)
        nc.vector.tensor_tensor(z2m, z2, fbig[:NP, :], Alu.max)
        zi = z2m.bitcast(i32)
        # monotone signed-int mapping of floats: key = zi ^ ((zi >> 31) & 0x7fffffff)
        t1 = kpool.tile([NP, 128], i32)
        nc.vector.tensor_tensor(
            t1, zi, ci[:NP, 0:1].to_broadcast([NP, 128]), Alu.arith_shift_right
        )
        t2 = kpool.tile([NP, 128], i32)
        nc.vector.tensor_tensor(
            t2, t1, ci[:NP, 1:2].to_broadcast([NP, 128]), Alu.bitwise_and
        )
        key = kpool.tile([NP, 128], i32)
        nc.vector.tensor_tensor(key, zi, t2, Alu.bitwise_xor)

        # DVE comparisons run in the fp32 ALU, so split the 32-bit key into a
        # hi-24-bit and lo-8-bit half, both exactly representable in fp32,
        # and do a two-level bitwise descent entirely in fp32.
        khi_i = kpool.tile([NP, 128], i32)
        nc.vector.tensor_tensor(
            khi_i, key, ci[:NP, 2:3].to_broadcast([NP, 128]), Alu.arith_shift_right
        )
        klo_i = kpool.tile([NP, 128], i32)
        nc.vector.tensor_tensor(
            klo_i, key, ci[:NP, 3:4].to_broadcast([NP, 128]), Alu.bitwise_and
        )
        khi = kpool.tile([NP, 128], f32)
        nc.vector.tensor_copy(khi, khi_i)
        klo = kpool.tile([NP, 128], f32)
        nc.vector.tensor_copy(klo, klo_i)

        def count_descent(vals, mask, base, nbits, lo_init, label):
            """fp32 bitwise descent: returns max{v : base + #(mask & vals<=v) < kp}
            as a [NP,1] fp32 tile (mask/base optional)."""
            lo_t = kpool.tile([NP, 1], f32, name=f"lo{label}")
            nc.vector.memset(lo_t, float(lo_init))
            for b in range(nbits - 1, -1, -1):
                cand = kpool.tile([NP, 1], f32, tag="cand", name=f"cand{label}_{b}")
                nc.vector.tensor_scalar(cand, lo_t, float(2 ** b), None, Alu.add)
                lef = kpool.tile([NP, 128], f32, tag="lef", name=f"lef{label}_{b}")
                if mask is None:
                    nc.vector.tensor_scalar(lef, vals, cand, None, Alu.is_le)
                else:
                    nc.vector.scalar_tensor_tensor(
                        lef, vals, cand, mask, Alu.is_le, Alu.mult
                    )
                rs = kpool.tile([NP, 1], f32, tag="rs", name=f"rs{label}_{b}")
                nc.vector.tensor_reduce(rs, lef, AxX, Alu.add)
                cs = psK.tile([NP, 1], f32, tag="cs", name=f"cs{label}_{b}")
                nc.tensor.matmul(cs, hmat[:NP, :NP], rs, start=True, stop=True)
                cond = kpool.tile([NP, 1], f32, tag="cond", name=f"cond{label}_{b}")
                if base is None:
                    nc.vector.tensor_scalar(cond, cs, float(kp), None, Alu.is_lt)
                else:
                    nc.vector.tensor_scalar(cond, cs, base, float(kp), Alu.add, Alu.is_lt)
                step = kpool.tile([NP, 1], f32, tag="step", name=f"step{label}_{b}")
                nc.vector.tensor_scalar(step, cond, float(2 ** b), None, Alu.mult)
                nc.vector.tensor_tensor(lo_t, lo_t, step, Alu.add)
            res = kpool.tile([NP, 1], f32, name=f"thr{label}")
            nc.vector.tensor_scalar(res, lo_t, 1.0, None, Alu.add)
            return res

        # level 1: hi-part threshold H* (khi in [-2^23, 2^23))
        hstar = count_descent(khi, None, None, 24, -(2 ** 23) - 1, "H")
        lessH = kpool.tile([NP, 128], f32)
        nc.vector.tensor_scalar(lessH, khi, hstar, None, Alu.is_lt)
        eqH = kpool.tile([NP, 128], f32)
        nc.vector.tensor_scalar(eqH, khi, hstar, None, Alu.is_equal)
        rbh = kpool.tile([NP, 1], f32)
        nc.vector.tensor_reduce(rbh, lessH, AxX, Alu.add)
        cbp = psK.tile([NP, 1], f32, tag="cs", name="cbp")
        nc.tensor.matmul(cbp, hmat[:NP, :NP], rbh, start=True, stop=True)
        cbase = kpool.tile([NP, 1], f32)
        nc.vector.tensor_copy(cbase, cbp)
        # level 2: lo-part threshold among khi == H* (klo in [0, 256))
        lstar = count_descent(klo, eqH, cbase, 8, -1.0, "L")

        less = kpool.tile([NP, 128], f32)
        nc.vector.scalar_tensor_tensor(less, klo, lstar, eqH, Alu.is_lt, Alu.mult)
        nc.vector.tensor_tensor(less, less, lessH, Alu.add)
        eqf = kpool.tile([NP, 128], f32)
        nc.vector.scalar_tensor_tensor(eqf, klo, lstar, eqH, Alu.is_equal, Alu.mult)

        rl = kpool.tile([NP, 1], f32)
        nc.vector.tensor_reduce(rl, less, AxX, Alu.add)
        cls = psK.tile([NP, 1], f32, tag="cs", name="cls")
        nc.tensor.matmul(cls, hmat[:NP, :NP], rl, start=True, stop=True)
        clessf = kpool.tile([NP, 1], f32)
        nc.vector.tensor_copy(clessf, cls)

        re_ = kpool.tile([NP, 1], f32)
        nc.vector.tensor_reduce(re_, eqf, AxX, Alu.add)
        eqo = psK.tile([NP, 1], f32, tag="cs", name="eqo")
        nc.tensor.matmul(eqo, texcl[:NP, :NP], re_, start=True, stop=True)
        eqof = kpool.tile([NP, 1], f32)
        nc.vector.tensor_copy(eqof, eqo)

        # within-partition exclusive prefix of eq along f (log-shift adds)
        pa = kpool.tile([NP, 192], f32)
        pb = kpool.tile([NP, 192], f32)
        nc.vector.memset(pa[:, 0:64], 0.0)
        nc.vector.memset(pb[:, 0:64], 0.0)
        nc.vector.tensor_copy(pa[:, 64:192], eqf)
        cur, nxt = pa, pb
        for sh in (1, 2, 4, 8, 16, 32, 64):
            nc.vector.tensor_tensor(
                nxt[:, 64:192], cur[:, 64:192], cur[:, 64 - sh : 192 - sh], Alu.add
            )
            cur, nxt = nxt, cur
        excl = kpool.tile([NP, 128], f32)
        nc.vector.tensor_tensor(excl, cur[:, 64:192], eqf, Alu.subtract)
        exoff = kpool.tile([NP, 128], f32)
        nc.vector.tensor_scalar(exoff, excl, eqof, None, Alu.add)
        # keep the tied element iff (#ties before it) + c_less < kp
        kec = kpool.tile([NP, 128], f32)
        nc.vector.tensor_scalar(kec, exoff, clessf, float(kp), Alu.add, Alu.is_lt)
        keq = kpool.tile([NP, 128], f32)
        nc.vector.tensor_tensor(keq, kec, eqf, Alu.mult)
        keep = kpool.tile([NP, 128], f32)
        nc.vector.tensor_tensor(keep, less, keq, Alu.add)
        nc.vector.tensor_tensor(keep, keep, fone[:NP, :], Alu.add)
        yh8 = kpool.tile([NP, 128], u8)
        nc.vector.tensor_copy(yh8, keep)

        ysf = kpool.tile([NP, 128], f32)
        nc.scalar.activation(ysf, z2, Act.Sigmoid)

        nc.sync.dma_start(out=ys_d[:].rearrange("(p f) -> p f", f=128), in_=ysf)
        nc.sync.dma_start(out=yh_d[:].rearrange("(p f) -> p f", f=128), in_=yh8)

    nc.compile()
    return nc


def _host_consts(S, D, W3):
    """Host-precomputed constant tensors shared by all cores."""
    SB = S // 128
    KC = D // 128
    ident = np.eye(128, dtype=np.float32)
    blk = np.arange(128) // SB
    hmat = (blk[:, None] == blk[None, :]).astype(np.float32)
    iidx = np.arange(128)
    texcl = ((blk[:, None] == blk[None, :]) & (iidx[:, None] < iidx[None, :])).astype(
        np.float32
    )
    first = (np.arange(128) % SB == 0)[:, None] & (np.arange(128) == 0)[None, :]
    fbig = np.where(first, np.float32(3.0e38), np.float32(-3.0e38)).astype(np.float32)
    fone = first.astype(np.float32)
    vals = [31, 0x7FFFFFFF, 8, 0xFF]
    ci = np.tile(np.array(vals, dtype=np.int64).astype(np.int32), (128, 1))
    w3e = np.zeros((128, KC, 4, 4), dtype=np.float32)
    w3r = np.asarray(W3, dtype=np.float32).reshape(KC, 128).T  # [p, kc]
    for s in range(4):
        w3e[:, :, s, s] = w3r
    w3e = np.ascontiguousarray(w3e.reshape(128, KC * 16))
    return ident, hmat, texcl, fbig, fone, ci, w3e


_NC_CACHE = {}

# test-harness hooks: set TRACE=True before calling kernel() to profile; the
# BassKernelResults of the last run (with exec_time_ns when traced) lands in
# LAST_RESULTS.
TRACE = False
LAST_RESULTS = None


def kernel(inputs, W1, b1, W2, b2, W3, b3, k):
    from concourse.bass_utils import run_bass_kernel_spmd

    x = np.asarray(inputs, dtype=np.float32)
    W1 = np.asarray(W1, dtype=np.float32)
    W2 = np.asarray(W2, dtype=np.float32)
    W3 = np.asarray(W3, dtype=np.float32)
    b1 = np.asarray(b1, dtype=np.float32)
    b2 = np.asarray(b2, dtype=np.float32)
    b3 = np.asarray(b3, dtype=np.float32)
    k = int(k)

    Bp, Sp, Dp = x.shape
    b_local = Bp // N_CORES
    T = b_local * Sp
    has_b1 = bool(np.any(b1 != 0))
    has_b2 = bool(np.any(b2 != 0))
    b3val = float(b3.reshape(-1)[0])

    ckey = (b_local, Sp, Dp, k, has_b1, has_b2, b3val)
    if ckey not in _NC_CACHE:
        _NC_CACHE[ckey] = _build_nc(b_local, Sp, Dp, k, has_b1, has_b2, b3val)
    nc = _NC_CACHE[ckey]

    ident, hmat, texcl, fbig, fone, ci, w3e = _host_consts(Sp, Dp, W3)
    w1t = np.ascontiguousarray(W1.T)
    w2t = np.ascontiguousarray(W2.T)

    shared = {
        "w1t": w1t, "w2t": w2t, "w3e": w3e, "ident": ident, "hmat": hmat,
        "texcl": texcl, "fbig": fbig, "fone": fone, "ci": ci,
    }
    if has_b1:
        shared["b1r"] = np.ascontiguousarray(b1.reshape(1, Dp))
    if has_b2:
        shared["b2r"] = np.ascontiguousarray(b2.reshape(1, Dp))
    if has_b1 or has_b2:
        shared["ones"] = np.ones((1, 512), dtype=np.float32)

    in_maps = []
    for c in range(N_CORES):
        shard = np.ascontiguousarray(
            x[c * b_local : (c + 1) * b_local].reshape(T, Dp)
        )
        in_maps.append({"x": shard, **shared})

    global LAST_RESULTS
    res = run_bass_kernel_spmd(
        nc, in_maps, core_ids=list(range(N_CORES)), trace=TRACE
    )
    LAST_RESULTS = res
    y_soft = np.concatenate(
        [r["y_soft"].reshape(b_local, Sp) for r in res.results], axis=0
    )
    y_hard = np.concatenate(
        [r["y_hard"].reshape(b_local, Sp) for r in res.results], axis=0
    ).astype(bool)
    return (y_hard, y_soft)


# revision 27
# speedup vs baseline: 35.5388x; 35.5388x over previous
"""Trainium2 Bass kernel for nn_BERTStudentPruner.

Computation (per the reference):
    h1 = relu(x @ W1.T + b1)            x: [B, S, D]
    h2 = relu(h1 @ W2.T + b2)
    y_soft = sigmoid(h2 @ W3.T + b3)    -> [B, S]
    y_hard = rank(y_soft, per-row stable ascending) < k, with token 0 of each
             row forced below the min so it is always kept.

Strategy: pure data parallelism over the batch dim (8 batches per core on 8
NeuronCores).  Per core, a fully-fused fp32 MLP (PE transposes the activation
tiles; weights are host-pre-transposed) followed by an exact on-device top-k
mask built from a 32-step bitwise binary search over the monotone int32
mapping of the logits (ties broken by index via a prefix-sum, exactly
matching stable argsort semantics).
"""

import sys

for _p in ("/opt/trn_rl_repo",):
    if _p not in sys.path:
        sys.path.insert(0, _p)

import numpy as np

B, S, D = 64, 2048, 768
N_CORES = 8
B_LOCAL = B // N_CORES


def _build_nc(B_local, S, D, k, has_b1, has_b2, b3val, fp16=False):
    from contextlib import ExitStack

    import concourse.bass as bass
    import concourse.bacc as bacc
    import concourse.mybir as mybir
    import concourse.tile as tile

    f32 = mybir.dt.float32
    f16 = mybir.dt.float16
    i32 = mybir.dt.int32
    u8 = mybir.dt.uint8
    Alu = mybir.AluOpType
    Act = mybir.ActivationFunctionType
    AxX = mybir.AxisListType.X

    T = B_local * S          # tokens per core
    KC = D // 128            # feature chunks
    NG = T // 512            # 512-token groups
    NP = T // 128            # partitions used in the [NP, 128] token layout
    kp = k - 1               # slots left after the always-kept token 0
    assert T % 512 == 0 and D % 128 == 0 and NP <= 128 and S % 128 == 0

    nc = bacc.Bacc()
    x_d = nc.declare_dram_parameter("x", [T, D], f32, isOutput=False)
    if fp16:
        w1h_d = nc.declare_dram_parameter("w1h", [D, D], f16, isOutput=False)
        w1l_d = nc.declare_dram_parameter("w1l", [D, D], f16, isOutput=False)
        w2h_d = nc.declare_dram_parameter("w2h", [D, D], f16, isOutput=False)
        w2l_d = nc.declare_dram_parameter("w2l", [D, D], f16, isOutput=False)
    else:
        w1_d = nc.declare_dram_parameter("w1t", [D, D], f32, isOutput=False)
        w2_d = nc.declare_dram_parameter("w2t", [D, D], f32, isOutput=False)
    w3_d = nc.declare_dram_parameter("w3e", [128, KC * 16], f32, isOutput=False)
    wdt = f16 if fp16 else f32
    id_d = nc.declare_dram_parameter("ident", [128, 128], wdt, isOutput=False)
    hm_d = nc.declare_dram_parameter("hmat", [128, 128], f32, isOutput=False)
    tx_d = nc.declare_dram_parameter("texcl", [128, 128], f32, isOutput=False)
    fb_d = nc.declare_dram_parameter("fbig", [128, 128], f32, isOutput=False)
    fo_d = nc.declare_dram_parameter("fone", [128, 128], f32, isOutput=False)
    ci_d = nc.declare_dram_parameter("ci", [128, 4], i32, isOutput=False)
    if has_b1:
        b1_d = nc.declare_dram_parameter("b1r", [1, D], f32, isOutput=False)
    if has_b2:
        b2_d = nc.declare_dram_parameter("b2r", [1, D], f32, isOutput=False)
    if has_b1 or has_b2:
        on_d = nc.declare_dram_parameter("ones", [1, 512], f32, isOutput=False)
    ys_d = nc.declare_dram_parameter("y_soft", [T], f32, isOutput=True)
    yh_d = nc.declare_dram_parameter("y_hard", [T], u8, isOutput=True)

    with ExitStack() as ctx:
        tc = ctx.enter_context(tile.TileContext(nc))
        cpool = ctx.enter_context(tc.tile_pool(name="consts", bufs=1))

        if fp16:
            w1hsb = cpool.tile([128, KC, D], f16)
            nc.sync.dma_start(out=w1hsb, in_=w1h_d[:].rearrange("(kc p) e -> p kc e", p=128))
            w1lsb = cpool.tile([128, KC, D], f16)
            nc.sync.dma_start(out=w1lsb, in_=w1l_d[:].rearrange("(kc p) e -> p kc e", p=128))
            w2hsb = cpool.tile([128, KC, D], f16)
            nc.sync.dma_start(out=w2hsb, in_=w2h_d[:].rearrange("(kc p) e -> p kc e", p=128))
            w2lsb = cpool.tile([128, KC, D], f16)
            nc.sync.dma_start(out=w2lsb, in_=w2l_d[:].rearrange("(kc p) e -> p kc e", p=128))
        else:
            w1sb = cpool.tile([128, KC, D], f32)
            nc.sync.dma_start(out=w1sb, in_=w1_d[:].rearrange("(kc p) e -> p kc e", p=128))
            w2sb = cpool.tile([128, KC, D], f32)
            nc.sync.dma_start(out=w2sb, in_=w2_d[:].rearrange("(kc p) e -> p kc e", p=128))
        w3sb = cpool.tile([128, KC, 4, 4], f32)
        nc.sync.dma_start(out=w3sb, in_=w3_d[:].rearrange("p (kc s m) -> p kc s m", s=4, m=4))
        ident = cpool.tile([128, 128], wdt)
        nc.sync.dma_start(out=ident, in_=id_d[:])
        hmat = cpool.tile([128, 128], f32)
        nc.sync.dma_start(out=hmat, in_=hm_d[:])
        texcl = cpool.tile([128, 128], f32)
        nc.sync.dma_start(out=texcl, in_=tx_d[:])
        fbig = cpool.tile([128, 128], f32)
        nc.sync.dma_start(out=fbig, in_=fb_d[:])
        fone = cpool.tile([128, 128], f32)
        nc.sync.dma_start(out=fone, in_=fo_d[:])
        ci = cpool.tile([128, 4], i32)
        nc.sync.dma_start(out=ci, in_=ci_d[:])
        if has_b1:
            b1sb = cpool.tile([1, D], f32)
            nc.sync.dma_start(out=b1sb, in_=b1_d[:])
        if has_b2:
            b2sb = cpool.tile([1, D], f32)
            nc.sync.dma_start(out=b2sb, in_=b2_d[:])
        if has_b1 or has_b2:
            onesb = cpool.tile([1, 512], f32)
            nc.sync.dma_start(out=onesb, in_=on_d[:])

        z2 = cpool.tile([NP, 128], f32)

        xpool = ctx.enter_context(tc.tile_pool(name="xn", bufs=2))
        xtpool = ctx.enter_context(tc.tile_pool(name="xt", bufs=2))
        h1pool = ctx.enter_context(tc.tile_pool(name="h1", bufs=2))
        h2pool = ctx.enter_context(tc.tile_pool(name="h2", bufs=2))
        psT = ctx.enter_context(tc.tile_pool(name="psT", bufs=2, space="PSUM"))
        psMM = ctx.enter_context(
            tc.tile_pool(name="psMM", bufs=(2 if fp16 else 3), space="PSUM")
        )
        psZ = ctx.enter_context(
            tc.tile_pool(name="psZ", bufs=(1 if fp16 else 2), space="PSUM")
        )
        zrpool = ctx.enter_context(tc.tile_pool(name="zr", bufs=2))

        SPLIT_SCALE = 2048.0  # 2^11: exact fp16 residual scaling

        for g in range(NG):
            xn = xpool.tile([128, 4, D], f32, tag="xn", name=f"xn{g}")
            nc.sync.dma_start(
                out=xn,
                in_=x_d[g * 512 : (g + 1) * 512, :].rearrange("(s p) e -> p s e", p=128),
            )
            if fp16:
                # split x into fp16 hi + scaled fp16 residual, then transpose
                # both halves on the PE (fp16 transposes run at full rate)
                xnh = xpool.tile([128, 4, D], f16, tag="xnh", name=f"xnh{g}")
                nc.vector.tensor_copy(xnh, xn)
                xnd = xpool.tile([128, 4, D], f32, tag="xnd", name=f"xnd{g}")
                nc.vector.tensor_tensor(xnd, xn, xnh, Alu.subtract)
                xnl = xpool.tile([128, 4, D], f16, tag="xnl", name=f"xnl{g}")
                nc.vector.tensor_scalar(xnl, xnd, SPLIT_SCALE, None, Alu.mult)
                xths, xtls = [], []
                for kc in range(KC):
                    ptg = psT.tile([128, 1024], f16, tag="pt", name=f"pt{g}_{kc}")
                    for s in range(4):
                        nc.tensor.transpose(
                            ptg[:, s * 128 : (s + 1) * 128],
                            xnh[:, s, kc * 128 : (kc + 1) * 128],
                            ident,
                        )
                        nc.tensor.transpose(
                            ptg[:, 512 + s * 128 : 512 + (s + 1) * 128],
                            xnl[:, s, kc * 128 : (kc + 1) * 128],
                            ident,
                        )
                    xth = xtpool.tile([128, 512], f16, tag=f"xth{kc}", name=f"xth{g}_{kc}")
                    nc.vector.tensor_copy(xth, ptg[:, 0:512])
                    xtl = xtpool.tile([128, 512], f16, tag=f"xtl{kc}", name=f"xtl{g}_{kc}")
                    nc.vector.tensor_copy(xtl, ptg[:, 512:1024])
                    xths.append(xth)
                    xtls.append(xtl)

                def fp16_layer(g, lname, whsb, wlsb, rhs_h, rhs_l, bsb, has_b, hpool,
                               split_out):
                    outs = []
                    for ec in range(KC):
                        psa = psMM.tile([128, 512], f32, tag="mmA", name=f"{lname}A_{g}_{ec}")
                        psb = psMM.tile([128, 512], f32, tag="mmB", name=f"{lname}B_{g}_{ec}")
                        for kc in range(KC):
                            wh = whsb[:, kc, ec * 128 : (ec + 1) * 128]
                            wl = wlsb[:, kc, ec * 128 : (ec + 1) * 128]
                            nc.tensor.matmul(psa, wh, rhs_h[kc], start=(kc == 0),
                                             stop=(kc == KC - 1 and not has_b))
                            nc.tensor.matmul(psb, wl, rhs_h[kc], start=(kc == 0),
                                             stop=False)
                            nc.tensor.matmul(psb, wh, rhs_l[kc], start=False,
                                             stop=(kc == KC - 1))
                        if has_b:
                            nc.tensor.matmul(psa, bsb[:, ec * 128 : (ec + 1) * 128],
                                             onesb, start=False, stop=True)
                        hcmb = hpool.tile([128, 512], f32, tag=f"hc{lname}{ec}",
                                          name=f"hc{lname}_{g}_{ec}")
                        nc.vector.scalar_tensor_tensor(
                            hcmb, psb, 1.0 / SPLIT_SCALE, psa, Alu.mult, Alu.add
                        )
                        h = hpool.tile([128, 512], f32, tag=f"h{lname}{ec}",
                                       name=f"h{lname}_{g}_{ec}")
                        nc.scalar.activation(h, hcmb, Act.Relu)
                        if split_out:
                            hh = hpool.tile([128, 512], f16, tag=f"hh{lname}{ec}",
                                            name=f"hh{lname}_{g}_{ec}")
                            nc.vector.tensor_copy(hh, h)
                            hd = hpool.tile([128, 512], f32, tag=f"hd{lname}{ec}",
                                            name=f"hd{lname}_{g}_{ec}")
                            nc.vector.tensor_tensor(hd, h, hh, Alu.subtract)
                            hl = hpool.tile([128, 512], f16, tag=f"hl{lname}{ec}",
                                            name=f"hl{lname}_{g}_{ec}")
                            nc.vector.tensor_scalar(hl, hd, SPLIT_SCALE, None, Alu.mult)
                            outs.append((h, hh, hl))
                        else:
                            outs.append((h, None, None))
                    return outs

                l1 = fp16_layer(g, "1", w1hsb, w1lsb, xths, xtls,
                                b1sb if has_b1 else None, has_b1, h1pool, True)
                h1hs = [t[1] for t in l1]
                h1ls = [t[2] for t in l1]
                l2 = fp16_layer(g, "2", w2hsb, w2lsb, h1hs, h1ls,
                                b2sb if has_b2 else None, has_b2, h2pool, False)
                h2s = [t[0] for t in l2]
            else:
                # transpose x tiles: [token, d] -> [d, token]
                xts = []
                for kc in range(KC):
                    xt = xtpool.tile([128, 512], f32, tag=f"xt{kc}", name=f"xt{g}_{kc}")
                    ptg = psT.tile([128, 512], f32, tag="pt", name=f"pt{g}_{kc}")
                    for s in range(4):
                        nc.tensor.transpose(
                            ptg[:, s * 128 : (s + 1) * 128],
                            xn[:, s, kc * 128 : (kc + 1) * 128],
                            ident,
                        )
                    nc.vector.tensor_copy(xt, ptg)
                    xts.append(xt)
                # layer 1
                h1s = []
                for ec in range(KC):
                    ps = psMM.tile([128, 512], f32, tag="mm", name=f"p1_{g}_{ec}")
                    for kc in range(KC):
                        nc.tensor.matmul(
                            ps,
                            w1sb[:, kc, ec * 128 : (ec + 1) * 128],
                            xts[kc],
                            start=(kc == 0),
                            stop=(kc == KC - 1 and not has_b1),
                        )
                    if has_b1:
                        nc.tensor.matmul(
                            ps, b1sb[:, ec * 128 : (ec + 1) * 128], onesb,
                            start=False, stop=True,
                        )
                    h1 = h1pool.tile([128, 512], f32, tag=f"h1{ec}", name=f"h1_{g}_{ec}")
                    nc.scalar.activation(h1, ps, Act.Relu)
                    h1s.append(h1)
                # layer 2
                h2s = []
                for fc in range(KC):
                    ps = psMM.tile([128, 512], f32, tag="mm", name=f"p2_{g}_{fc}")
                    for kc in range(KC):
                        nc.tensor.matmul(
                            ps,
                            w2sb[:, kc, fc * 128 : (fc + 1) * 128],
                            h1s[kc],
                            start=(kc == 0),
                            stop=(kc == KC - 1 and not has_b2),
                        )
                    if has_b2:
                        nc.tensor.matmul(
                            ps, b2sb[:, fc * 128 : (fc + 1) * 128], onesb,
                            start=False, stop=True,
                        )
                    h2 = h2pool.tile([128, 512], f32, tag=f"h2{fc}", name=f"h2_{g}_{fc}")
                    nc.scalar.activation(h2, ps, Act.Relu)
                    h2s.append(h2)
            # layer 3: z[t] = w3 . h2[t]; routed so subtile s lands on psum
            # partition s (lhsT column s holds w3, other columns zero)
            z4 = psZ.tile([4, 128], f32, tag="z4", name=f"z4_{g}")
            first = True
            for kc in range(KC):
                for s in range(4):
                    nc.tensor.matmul(
                        z4,
                        w3sb[:, kc, s, :],
                        h2s[kc][:, s * 128 : (s + 1) * 128],
                        start=first,
                        stop=(kc == KC - 1 and s == 3),
                    )
                    first = False
            zrow = zrpool.tile([4, 128], f32, tag="zr", name=f"zr{g}")
            nc.scalar.activation(zrow, z4, Act.Copy, bias=float(b3val))
            nc.sync.dma_start(out=z2[4 * g : 4 * g + 4, :], in_=zrow)

        # ---- top-k mask on z2 [NP, 128]; token t = 128*p + f ----
        kpool = ctx.enter_context(tc.tile_pool(name="kp", bufs=1))
        psK = ctx.enter_context(tc.tile_pool(name="psK", bufs=1, space="PSUM"))

        z2m = kpool.tile([NP, 128], f32)
        nc.vector.tensor_tensor(z2m, z2, fbig[:NP, :], Alu.max)
        zi = z2m.bitcast(i32)
        # monotone signed-int mapping of floats: key = zi ^ ((zi >> 31) & 0x7fffffff)
        t1 = kpool.tile([NP, 128], i32)
        nc.vector.tensor_tensor(
            t1, zi, ci[:NP, 0:1].to_broadcast([NP, 128]), Alu.arith_shift_right
        )
        t2 = kpool.tile([NP, 128], i32)
        nc.vector.tensor_tensor(
            t2, t1, ci[:NP, 1:2].to_broadcast([NP, 128]), Alu.bitwise_and
        )
        key = kpool.tile([NP, 128], i32)
        nc.vector.tensor_tensor(key, zi, t2, Alu.bitwise_xor)

        # DVE comparisons run in the fp32 ALU, so split the 32-bit key into a
        # hi-24-bit and lo-8-bit half, both exactly representable in fp32,
        # and do a two-level bitwise descent entirely in fp32.
        khi_i = kpool.tile([NP, 128], i32)
        nc.vector.tensor_tensor(
            khi_i, key, ci[:NP, 2:3].to_broadcast([NP, 128]), Alu.arith_shift_right
        )
        klo_i = kpool.tile([NP, 128], i32)
        nc.vector.tensor_tensor(
            klo_i, key, ci[:NP, 3:4].to_broadcast([NP, 128]), Alu.bitwise_and
        )
        khi = kpool.tile([NP, 128], f32)
        nc.vector.tensor_copy(khi, khi_i)
        klo = kpool.tile([NP, 128], f32)
        nc.vector.tensor_copy(klo, klo_i)

        def count_descent(vals, mask, base, nbits, lo_init, label):
            """fp32 bitwise descent: returns max{v : base + #(mask & vals<=v) < kp}
            as a [NP,1] fp32 tile (mask/base optional)."""
            lo_t = kpool.tile([NP, 1], f32, name=f"lo{label}")
            nc.vector.memset(lo_t, float(lo_init))
            for b in range(nbits - 1, -1, -1):
                cand = kpool.tile([NP, 1], f32, tag="cand", name=f"cand{label}_{b}")
                nc.vector.tensor_scalar(cand, lo_t, float(2 ** b), None, Alu.add)
                lef = kpool.tile([NP, 128], f32, tag="lef", name=f"lef{label}_{b}")
                if mask is None:
                    nc.vector.tensor_scalar(lef, vals, cand, None, Alu.is_le)
                else:
                    nc.vector.scalar_tensor_tensor(
                        lef, vals, cand, mask, Alu.is_le, Alu.mult
                    )
                rs = kpool.tile([NP, 1], f32, tag="rs", name=f"rs{label}_{b}")
                nc.vector.tensor_reduce(rs, lef, AxX, Alu.add)
                cs = psK.tile([NP, 1], f32, tag="cs", name=f"cs{label}_{b}")
                nc.tensor.matmul(cs, hmat[:NP, :NP], rs, start=True, stop=True)
                cond = kpool.tile([NP, 1], f32, tag="cond", name=f"cond{label}_{b}")
                if base is None:
                    nc.vector.tensor_scalar(cond, cs, float(kp), None, Alu.is_lt)
                else:
                    nc.vector.tensor_scalar(cond, cs, base, float(kp), Alu.add, Alu.is_lt)
                step = kpool.tile([NP, 1], f32, tag="step", name=f"step{label}_{b}")
                nc.vector.tensor_scalar(step, cond, float(2 ** b), None, Alu.mult)
                nc.vector.tensor_tensor(lo_t, lo_t, step, Alu.add)
            res = kpool.tile([NP, 1], f32, name=f"thr{label}")
            nc.vector.tensor_scalar(res, lo_t, 1.0, None, Alu.add)
            return res

        # level 1: hi-part threshold H* (khi in [-2^23, 2^23))
        hstar = count_descent(khi, None, None, 24, -(2 ** 23) - 1, "H")
        lessH = kpool.tile([NP, 128], f32)
        nc.vector.tensor_scalar(lessH, khi, hstar, None, Alu.is_lt)
        eqH = kpool.tile([NP, 128], f32)
        nc.vector.tensor_scalar(eqH, khi, hstar, None, Alu.is_equal)
        rbh = kpool.tile([NP, 1], f32)
        nc.vector.tensor_reduce(rbh, lessH, AxX, Alu.add)
        cbp = psK.tile([NP, 1], f32, tag="cs", name="cbp")
        nc.tensor.matmul(cbp, hmat[:NP, :NP], rbh, start=True, stop=True)
        cbase = kpool.tile([NP, 1], f32)
        nc.vector.tensor_copy(cbase, cbp)
        # level 2: lo-part threshold among khi == H* (klo in [0, 256))
        lstar = count_descent(klo, eqH, cbase, 8, -1.0, "L")

        less = kpool.tile([NP, 128], f32)
        nc.vector.scalar_tensor_tensor(less, klo, lstar, eqH, Alu.is_lt, Alu.mult)
        nc.vector.tensor_tensor(less, less, lessH, Alu.add)
        eqf = kpool.tile([NP, 128], f32)
        nc.vector.scalar_tensor_tensor(eqf, klo, lstar, eqH, Alu.is_equal, Alu.mult)

        rl = kpool.tile([NP, 1], f32)
        nc.vector.tensor_reduce(rl, less, AxX, Alu.add)
        cls = psK.tile([NP, 1], f32, tag="cs", name="cls")
        nc.tensor.matmul(cls, hmat[:NP, :NP], rl, start=True, stop=True)
        clessf = kpool.tile([NP, 1], f32)
        nc.vector.tensor_copy(clessf, cls)

        re_ = kpool.tile([NP, 1], f32)
        nc.vector.tensor_reduce(re_, eqf, AxX, Alu.add)
        eqo = psK.tile([NP, 1], f32, tag="cs", name="eqo")
        nc.tensor.matmul(eqo, texcl[:NP, :NP], re_, start=True, stop=True)
        eqof = kpool.tile([NP, 1], f32)
        nc.vector.tensor_copy(eqof, eqo)

        # within-partition exclusive prefix of eq along f (log-shift adds)
        pa = kpool.tile([NP, 192], f32)
        pb = kpool.tile([NP, 192], f32)
        nc.vector.memset(pa[:, 0:64], 0.0)
        nc.vector.memset(pb[:, 0:64], 0.0)
        nc.vector.tensor_copy(pa[:, 64:192], eqf)
        cur, nxt = pa, pb
        for sh in (1, 2, 4, 8, 16, 32, 64):
            nc.vector.tensor_tensor(
                nxt[:, 64:192], cur[:, 64:192], cur[:, 64 - sh : 192 - sh], Alu.add
            )
            cur, nxt = nxt, cur
        excl = kpool.tile([NP, 128], f32)
        nc.vector.tensor_tensor(excl, cur[:, 64:192], eqf, Alu.subtract)
        exoff = kpool.tile([NP, 128], f32)
        nc.vector.tensor_scalar(exoff, excl, eqof, None, Alu.add)
        # keep the tied element iff (#ties before it) + c_less < kp
        kec = kpool.tile([NP, 128], f32)
        nc.vector.tensor_scalar(kec, exoff, clessf, float(kp), Alu.add, Alu.is_lt)
        keq = kpool.tile([NP, 128], f32)
        nc.vector.tensor_tensor(keq, kec, eqf, Alu.mult)
        keep = kpool.tile([NP, 128], f32)
        nc.vector.tensor_tensor(keep, less, keq, Alu.add)
        nc.vector.tensor_tensor(keep, keep, fone[:NP, :], Alu.add)
        yh8 = kpool.tile([NP, 128], u8)
        nc.vector.tensor_copy(yh8, keep)

        ysf = kpool.tile([NP, 128], f32)
        nc.scalar.activation(ysf, z2, Act.Sigmoid)

        nc.sync.dma_start(out=ys_d[:].rearrange("(p f) -> p f", f=128), in_=ysf)
        nc.sync.dma_start(out=yh_d[:].rearrange("(p f) -> p f", f=128), in_=yh8)

    nc.compile()
    return nc


def _host_consts(S, D, W3):
    """Host-precomputed constant tensors shared by all cores."""
    SB = S // 128
    KC = D // 128
    ident = np.eye(128, dtype=np.float32)
    blk = np.arange(128) // SB
    hmat = (blk[:, None] == blk[None, :]).astype(np.float32)
    iidx = np.arange(128)
    texcl = ((blk[:, None] == blk[None, :]) & (iidx[:, None] < iidx[None, :])).astype(
        np.float32
    )
    first = (np.arange(128) % SB == 0)[:, None] & (np.arange(128) == 0)[None, :]
    fbig = np.where(first, np.float32(3.0e38), np.float32(-3.0e38)).astype(np.float32)
    fone = first.astype(np.float32)
    vals = [31, 0x7FFFFFFF, 8, 0xFF]
    ci = np.tile(np.array(vals, dtype=np.int64).astype(np.int32), (128, 1))
    w3e = np.zeros((128, KC, 4, 4), dtype=np.float32)
    w3r = np.asarray(W3, dtype=np.float32).reshape(KC, 128).T  # [p, kc]
    for s in range(4):
        w3e[:, :, s, s] = w3r
    w3e = np.ascontiguousarray(w3e.reshape(128, KC * 16))
    return ident, hmat, texcl, fbig, fone, ci, w3e


_NC_CACHE = {}

# test-harness hooks: set TRACE=True before calling kernel() to profile; the
# BassKernelResults of the last run (with exec_time_ns when traced) lands in
# LAST_RESULTS.
TRACE = False
TRACE_TMPDIR = None
LAST_RESULTS = None


_LDW_OPT_PATCHED = False


def _enable_ldw_opt():
    """walrus's redundant-LDWEIGHTS elimination is off by default in
    bass_utils; it is a ~16% win for this all-fp32 kernel (verified
    bit-identical outputs).  Patch the flag in; kernel() falls back to the
    stock flags if compilation fails."""
    global _LDW_OPT_PATCHED
    import concourse.bass_utils as bu

    if _LDW_OPT_PATCHED:
        return
    orig = bu.run_command

    def patched(argv, **kwargs):
        argv = [a.replace("--enable-ldw-opt=false", "--enable-ldw-opt=true")
                if isinstance(a, str) else a for a in argv]
        return orig(argv, **kwargs)

    bu.run_command = patched
    bu._ldw_opt_orig_run_command = orig
    _LDW_OPT_PATCHED = True


def _disable_ldw_opt():
    global _LDW_OPT_PATCHED
    import concourse.bass_utils as bu

    if _LDW_OPT_PATCHED and hasattr(bu, "_ldw_opt_orig_run_command"):
        bu.run_command = bu._ldw_opt_orig_run_command
        _LDW_OPT_PATCHED = False


def kernel(inputs, W1, b1, W2, b2, W3, b3, k):
    from concourse.bass_utils import run_bass_kernel_spmd

    x = np.asarray(inputs, dtype=np.float32)
    W1 = np.asarray(W1, dtype=np.float32)
    W2 = np.asarray(W2, dtype=np.float32)
    W3 = np.asarray(W3, dtype=np.float32)
    b1 = np.asarray(b1, dtype=np.float32)
    b2 = np.asarray(b2, dtype=np.float32)
    b3 = np.asarray(b3, dtype=np.float32)
    k = int(k)

    Bp, Sp, Dp = x.shape
    b_local = Bp // N_CORES
    T = b_local * Sp
    has_b1 = bool(np.any(b1 != 0))
    has_b2 = bool(np.any(b2 != 0))
    b3val = float(b3.reshape(-1)[0])

    ckey = (b_local, Sp, Dp, k, has_b1, has_b2, b3val)
    if ckey not in _NC_CACHE:
        _NC_CACHE[ckey] = _build_nc(b_local, Sp, Dp, k, has_b1, has_b2, b3val)
    nc = _NC_CACHE[ckey]

    ident, hmat, texcl, fbig, fone, ci, w3e = _host_consts(Sp, Dp, W3)
    w1t = np.ascontiguousarray(W1.T)
    w2t = np.ascontiguousarray(W2.T)

    shared = {
        "w1t": w1t, "w2t": w2t, "w3e": w3e, "ident": ident, "hmat": hmat,
        "texcl": texcl, "fbig": fbig, "fone": fone, "ci": ci,
    }
    if has_b1:
        shared["b1r"] = np.ascontiguousarray(b1.reshape(1, Dp))
    if has_b2:
        shared["b2r"] = np.ascontiguousarray(b2.reshape(1, Dp))
    if has_b1 or has_b2:
        shared["ones"] = np.ones((1, 512), dtype=np.float32)

    in_maps = []
    for c in range(N_CORES):
        shard = np.ascontiguousarray(
            x[c * b_local : (c + 1) * b_local].reshape(T, Dp)
        )
        in_maps.append({"x": shard, **shared})

    global LAST_RESULTS
    kw = {}
    if TRACE and TRACE_TMPDIR:
        kw["tmpdir"] = TRACE_TMPDIR
    _enable_ldw_opt()
    try:
        res = run_bass_kernel_spmd(
            nc, in_maps, core_ids=list(range(N_CORES)), trace=TRACE, **kw
        )
    except Exception:
        # fall back to stock walrus flags
        _disable_ldw_opt()
        res = run_bass_kernel_spmd(
            nc, in_maps, core_ids=list(range(N_CORES)), trace=TRACE, **kw
        )
    LAST_RESULTS = res
    y_soft = np.concatenate(
        [r["y_soft"].reshape(b_local, Sp) for r in res.results], axis=0
    )
    y_hard = np.concatenate(
        [r["y_hard"].reshape(b_local, Sp) for r in res.results], axis=0
    ).astype(bool)
    return (y_hard, y_soft)


# revision 34
# speedup vs baseline: 44.1355x; 1.2419x over previous
"""Trainium2 Bass kernel for nn_BERTStudentPruner.

Computation (per the reference):
    h1 = relu(x @ W1.T + b1)            x: [B, S, D]
    h2 = relu(h1 @ W2.T + b2)
    y_soft = sigmoid(h2 @ W3.T + b3)    -> [B, S]
    y_hard = rank(y_soft, per-row stable ascending) < k, with token 0 of each
             row forced below the min so it is always kept.

Strategy: pure data parallelism over the batch dim (8 batches per core on 8
NeuronCores).  Per core, a fully-fused fp32 MLP (PE transposes the activation
tiles; weights are host-pre-transposed) followed by an exact on-device top-k
mask built from a 32-step bitwise binary search over the monotone int32
mapping of the logits (ties broken by index via a prefix-sum, exactly
matching stable argsort semantics).
"""

import sys

for _p in ("/opt/trn_rl_repo",):
    if _p not in sys.path:
        sys.path.insert(0, _p)

import numpy as np

B, S, D = 64, 2048, 768
N_CORES = 8
B_LOCAL = B // N_CORES


def _build_nc(B_local, S, D, k, has_b1, has_b2, b3val, fp16=False):
    from contextlib import ExitStack

    import concourse.bass as bass
    import concourse.bacc as bacc
    import concourse.mybir as mybir
    import concourse.tile as tile

    f32 = mybir.dt.float32
    f16 = mybir.dt.float16
    i32 = mybir.dt.int32
    u8 = mybir.dt.uint8
    Alu = mybir.AluOpType
    Act = mybir.ActivationFunctionType
    AxX = mybir.AxisListType.X

    T = B_local * S          # tokens per core
    KC = D // 128            # feature chunks
    NG = T // 512            # 512-token groups
    NP = T // 128            # partitions used in the [NP, 128] token layout
    kp = k - 1               # slots left after the always-kept token 0
    assert T % 512 == 0 and D % 128 == 0 and NP <= 128 and S % 128 == 0

    nc = bacc.Bacc()
    x_d = nc.declare_dram_parameter("x", [T, D], f32, isOutput=False)
    if fp16:
        w1h_d = nc.declare_dram_parameter("w1h", [D, D], f16, isOutput=False)
        w1l_d = nc.declare_dram_parameter("w1l", [D, D], f16, isOutput=False)
        w2h_d = nc.declare_dram_parameter("w2h", [D, D], f16, isOutput=False)
        w2l_d = nc.declare_dram_parameter("w2l", [D, D], f16, isOutput=False)
    else:
        w1_d = nc.declare_dram_parameter("w1t", [D, D], f32, isOutput=False)
        w2_d = nc.declare_dram_parameter("w2t", [D, D], f32, isOutput=False)
    w3_d = nc.declare_dram_parameter("w3e", [128, KC * 16], f32, isOutput=False)
    wdt = f16 if fp16 else f32
    id_d = nc.declare_dram_parameter("ident", [128, 128], wdt, isOutput=False)
    hm_d = nc.declare_dram_parameter("hmat", [128, 128], f32, isOutput=False)
    tx_d = nc.declare_dram_parameter("texcl", [128, 128], f32, isOutput=False)
    fb_d = nc.declare_dram_parameter("fbig", [128, 128], f32, isOutput=False)
    fo_d = nc.declare_dram_parameter("fone", [128, 128], f32, isOutput=False)
    ci_d = nc.declare_dram_parameter("ci", [128, 4], i32, isOutput=False)
    if has_b1:
        b1_d = nc.declare_dram_parameter("b1r", [1, D], f32, isOutput=False)
    if has_b2:
        b2_d = nc.declare_dram_parameter("b2r", [1, D], f32, isOutput=False)
    if has_b1 or has_b2:
        on_d = nc.declare_dram_parameter("ones", [1, 512], f32, isOutput=False)
    ys_d = nc.declare_dram_parameter("y_soft", [T], f32, isOutput=True)
    yh_d = nc.declare_dram_parameter("y_hard", [T], u8, isOutput=True)

    with ExitStack() as ctx:
        tc = ctx.enter_context(tile.TileContext(nc))
        cpool = ctx.enter_context(tc.tile_pool(name="consts", bufs=1))

        if fp16:
            w1hsb = cpool.tile([128, KC, D], f16)
            nc.sync.dma_start(out=w1hsb, in_=w1h_d[:].rearrange("(kc p) e -> p kc e", p=128))
            w1lsb = cpool.tile([128, KC, D], f16)
            nc.sync.dma_start(out=w1lsb, in_=w1l_d[:].rearrange("(kc p) e -> p kc e", p=128))
            w2hsb = cpool.tile([128, KC, D], f16)
            nc.sync.dma_start(out=w2hsb, in_=w2h_d[:].rearrange("(kc p) e -> p kc e", p=128))
            w2lsb = cpool.tile([128, KC, D], f16)
            nc.sync.dma_start(out=w2lsb, in_=w2l_d[:].rearrange("(kc p) e -> p kc e", p=128))
        else:
            w1sb = cpool.tile([128, KC, D], f32)
            nc.sync.dma_start(out=w1sb, in_=w1_d[:].rearrange("(kc p) e -> p kc e", p=128))
            w2sb = cpool.tile([128, KC, D], f32)
            nc.sync.dma_start(out=w2sb, in_=w2_d[:].rearrange("(kc p) e -> p kc e", p=128))
        w3sb = cpool.tile([128, KC, 4, 4], f32)
        nc.sync.dma_start(out=w3sb, in_=w3_d[:].rearrange("p (kc s m) -> p kc s m", s=4, m=4))
        ident = cpool.tile([128, 128], wdt)
        nc.sync.dma_start(out=ident, in_=id_d[:])
        hmat = cpool.tile([128, 128], f32)
        nc.sync.dma_start(out=hmat, in_=hm_d[:])
        texcl = cpool.tile([128, 128], f32)
        nc.sync.dma_start(out=texcl, in_=tx_d[:])
        fbig = cpool.tile([128, 128], f32)
        nc.sync.dma_start(out=fbig, in_=fb_d[:])
        fone = cpool.tile([128, 128], f32)
        nc.sync.dma_start(out=fone, in_=fo_d[:])
        ci = cpool.tile([128, 4], i32)
        nc.sync.dma_start(out=ci, in_=ci_d[:])
        if has_b1:
            b1sb = cpool.tile([1, D], f32)
            nc.sync.dma_start(out=b1sb, in_=b1_d[:])
        if has_b2:
            b2sb = cpool.tile([1, D], f32)
            nc.sync.dma_start(out=b2sb, in_=b2_d[:])
        if has_b1 or has_b2:
            onesb = cpool.tile([1, 512], f32)
            nc.sync.dma_start(out=onesb, in_=on_d[:])

        z2 = cpool.tile([NP, 128], f32)

        xpool = ctx.enter_context(tc.tile_pool(name="xn", bufs=2))
        xtpool = ctx.enter_context(tc.tile_pool(name="xt", bufs=2))
        h1pool = ctx.enter_context(tc.tile_pool(name="h1", bufs=2))
        h2pool = ctx.enter_context(tc.tile_pool(name="h2", bufs=2))
        psT = ctx.enter_context(tc.tile_pool(name="psT", bufs=2, space="PSUM"))
        psMM = ctx.enter_context(
            tc.tile_pool(name="psMM", bufs=(2 if fp16 else 3), space="PSUM")
        )
        psZ = ctx.enter_context(
            tc.tile_pool(name="psZ", bufs=(1 if fp16 else 2), space="PSUM")
        )
        zrpool = ctx.enter_context(tc.tile_pool(name="zr", bufs=2))

        SPLIT_SCALE = 2048.0  # 2^11: exact fp16 residual scaling

        for g in range(NG):
            xn = xpool.tile([128, 4, D], f32, tag="xn", name=f"xn{g}")
            nc.sync.dma_start(
                out=xn,
                in_=x_d[g * 512 : (g + 1) * 512, :].rearrange("(s p) e -> p s e", p=128),
            )
            if fp16:
                # split x into fp16 hi + scaled fp16 residual, then transpose
                # both halves on the PE (fp16 transposes run at full rate)
                xnh = xpool.tile([128, 4, D], f16, tag="xnh", name=f"xnh{g}")
                nc.vector.tensor_copy(xnh, xn)
                xnd = xpool.tile([128, 4, D], f32, tag="xnd", name=f"xnd{g}", bufs=1)
                nc.vector.tensor_tensor(xnd, xn, xnh, Alu.subtract)
                xnl = xpool.tile([128, 4, D], f16, tag="xnl", name=f"xnl{g}")
                nc.vector.tensor_scalar(xnl, xnd, SPLIT_SCALE, None, Alu.mult)
                xths, xtls = [], []
                for kc in range(KC):
                    ptg = psT.tile([128, 1024], f16, tag="pt", name=f"pt{g}_{kc}")
                    for s in range(4):
                        nc.tensor.transpose(
                            ptg[:, s * 128 : (s + 1) * 128],
                            xnh[:, s, kc * 128 : (kc + 1) * 128],
                            ident,
                        )
                        nc.tensor.transpose(
                            ptg[:, 512 + s * 128 : 512 + (s + 1) * 128],
                            xnl[:, s, kc * 128 : (kc + 1) * 128],
                            ident,
                        )
                    xth = xtpool.tile([128, 512], f16, tag=f"xth{kc}", name=f"xth{g}_{kc}")
                    nc.vector.tensor_copy(xth, ptg[:, 0:512])
                    xtl = xtpool.tile([128, 512], f16, tag=f"xtl{kc}", name=f"xtl{g}_{kc}")
                    nc.vector.tensor_copy(xtl, ptg[:, 512:1024])
                    xths.append(xth)
                    xtls.append(xtl)

                def fp16_layer(g, lname, whsb, wlsb, rhs_h, rhs_l, bsb, has_b, hpool,
                               split_out):
                    outs = []
                    for ec in range(KC):
                        psa = psMM.tile([128, 512], f32, tag="mmA", name=f"{lname}A_{g}_{ec}")
                        psb = psMM.tile([128, 512], f32, tag="mmB", name=f"{lname}B_{g}_{ec}")
                        for kc in range(KC):
                            wh = whsb[:, kc, ec * 128 : (ec + 1) * 128]
                            wl = wlsb[:, kc, ec * 128 : (ec + 1) * 128]
                            nc.tensor.matmul(psa, wh, rhs_h[kc], start=(kc == 0),
                                             stop=(kc == KC - 1 and not has_b))
                            nc.tensor.matmul(psb, wl, rhs_h[kc], start=(kc == 0),
                                             stop=False)
                            nc.tensor.matmul(psb, wh, rhs_l[kc], start=False,
                                             stop=(kc == KC - 1))
                        if has_b:
                            nc.tensor.matmul(psa, bsb[:, ec * 128 : (ec + 1) * 128],
                                             onesb, start=False, stop=True)
                        hsc = hpool.tile([128, 512], f32, tag=f"hs{lname}",
                                         name=f"hs{lname}_{g}_{ec}", bufs=2)
                        nc.vector.tensor_scalar(hsc, psb, 1.0 / SPLIT_SCALE, None,
                                                Alu.mult)
                        hcmb = hpool.tile([128, 512], f32, tag=f"hc{lname}",
                                          name=f"hc{lname}_{g}_{ec}", bufs=2)
                        nc.vector.tensor_tensor(hcmb, hsc, psa, Alu.add)
                        if split_out:
                            h = hpool.tile([128, 512], f32, tag=f"h{lname}",
                                           name=f"h{lname}_{g}_{ec}", bufs=2)
                            nc.scalar.activation(h, hcmb, Act.Relu)
                            hh = hpool.tile([128, 512], f16, tag=f"hh{lname}{ec}",
                                            name=f"hh{lname}_{g}_{ec}")
                            nc.vector.tensor_copy(hh, h)
                            hd = hpool.tile([128, 512], f32, tag=f"hd{lname}",
                                            name=f"hd{lname}_{g}_{ec}", bufs=2)
                            nc.vector.tensor_tensor(hd, h, hh, Alu.subtract)
                            hl = hpool.tile([128, 512], f16, tag=f"hl{lname}{ec}",
                                            name=f"hl{lname}_{g}_{ec}")
                            nc.vector.tensor_scalar(hl, hd, SPLIT_SCALE, None, Alu.mult)
                            outs.append((h, hh, hl))
                        else:
                            h = hpool.tile([128, 512], f32, tag=f"h{lname}{ec}",
                                           name=f"h{lname}_{g}_{ec}")
                            nc.scalar.activation(h, hcmb, Act.Relu)
                            outs.append((h, None, None))
                    return outs

                l1 = fp16_layer(g, "1", w1hsb, w1lsb, xths, xtls,
                                b1sb if has_b1 else None, has_b1, h1pool, True)
                h1hs = [t[1] for t in l1]
                h1ls = [t[2] for t in l1]
                l2 = fp16_layer(g, "2", w2hsb, w2lsb, h1hs, h1ls,
                                b2sb if has_b2 else None, has_b2, h2pool, False)
                h2s = [t[0] for t in l2]
            else:
                # transpose x tiles: [token, d] -> [d, token]
                xts = []
                for kc in range(KC):
                    xt = xtpool.tile([128, 512], f32, tag=f"xt{kc}", name=f"xt{g}_{kc}")
                    ptg = psT.tile([128, 512], f32, tag="pt", name=f"pt{g}_{kc}")
                    for s in range(4):
                        nc.tensor.transpose(
                            ptg[:, s * 128 : (s + 1) * 128],
                            xn[:, s, kc * 128 : (kc + 1) * 128],
                            ident,
                        )
                    nc.vector.tensor_copy(xt, ptg)
                    xts.append(xt)
                # layer 1
                h1s = []
                for ec in range(KC):
                    ps = psMM.tile([128, 512], f32, tag="mm", name=f"p1_{g}_{ec}")
                    for kc in range(KC):
                        nc.tensor.matmul(
                            ps,
                            w1sb[:, kc, ec * 128 : (ec + 1) * 128],
                            xts[kc],
                            start=(kc == 0),
                            stop=(kc == KC - 1 and not has_b1),
                        )
                    if has_b1:
                        nc.tensor.matmul(
                            ps, b1sb[:, ec * 128 : (ec + 1) * 128], onesb,
                            start=False, stop=True,
                        )
                    h1 = h1pool.tile([128, 512], f32, tag=f"h1{ec}", name=f"h1_{g}_{ec}")
                    nc.scalar.activation(h1, ps, Act.Relu)
                    h1s.append(h1)
                # layer 2
                h2s = []
                for fc in range(KC):
                    ps = psMM.tile([128, 512], f32, tag="mm", name=f"p2_{g}_{fc}")
                    for kc in range(KC):
                        nc.tensor.matmul(
                            ps,
                            w2sb[:, kc, fc * 128 : (fc + 1) * 128],
                            h1s[kc],
                            start=(kc == 0),
                            stop=(kc == KC - 1 and not has_b2),
                        )
                    if has_b2:
                        nc.tensor.matmul(
                            ps, b2sb[:, fc * 128 : (fc + 1) * 128], onesb,
                            start=False, stop=True,
                        )
                    h2 = h2pool.tile([128, 512], f32, tag=f"h2{fc}", name=f"h2_{g}_{fc}")
                    nc.scalar.activation(h2, ps, Act.Relu)
                    h2s.append(h2)
            # layer 3: z[t] = w3 . h2[t]; routed so subtile s lands on psum
            # partition s (lhsT column s holds w3, other columns zero)
            z4 = psZ.tile([4, 128], f32, tag="z4", name=f"z4_{g}")
            first = True
            for kc in range(KC):
                for s in range(4):
                    nc.tensor.matmul(
                        z4,
                        w3sb[:, kc, s, :],
                        h2s[kc][:, s * 128 : (s + 1) * 128],
                        start=first,
                        stop=(kc == KC - 1 and s == 3),
                    )
                    first = False
            zrow = zrpool.tile([4, 128], f32, tag="zr", name=f"zr{g}")
            nc.scalar.activation(zrow, z4, Act.Copy, bias=float(b3val))
            nc.sync.dma_start(out=z2[4 * g : 4 * g + 4, :], in_=zrow)

        # ---- top-k mask on z2 [NP, 128]; token t = 128*p + f ----
        kpool = ctx.enter_context(tc.tile_pool(name="kp", bufs=1))
        psK = ctx.enter_context(tc.tile_pool(name="psK", bufs=1, space="PSUM"))

        z2m = kpool.tile([NP, 128], f32)
        nc.vector.tensor_tensor(z2m, z2, fbig[:NP, :], Alu.max)
        zi = z2m.bitcast(i32)
        # monotone signed-int mapping of floats: key = zi ^ ((zi >> 31) & 0x7fffffff)
        t1 = kpool.tile([NP, 128], i32)
        nc.vector.tensor_tensor(
            t1, zi, ci[:NP, 0:1].to_broadcast([NP, 128]), Alu.arith_shift_right
        )
        t2 = kpool.tile([NP, 128], i32)
        nc.vector.tensor_tensor(
            t2, t1, ci[:NP, 1:2].to_broadcast([NP, 128]), Alu.bitwise_and
        )
        key = kpool.tile([NP, 128], i32)
        nc.vector.tensor_tensor(key, zi, t2, Alu.bitwise_xor)

        # DVE comparisons run in the fp32 ALU, so split the 32-bit key into a
        # hi-24-bit and lo-8-bit half, both exactly representable in fp32,
        # and do a two-level bitwise descent entirely in fp32.
        khi_i = kpool.tile([NP, 128], i32)
        nc.vector.tensor_tensor(
            khi_i, key, ci[:NP, 2:3].to_broadcast([NP, 128]), Alu.arith_shift_right
        )
        klo_i = kpool.tile([NP, 128], i32)
        nc.vector.tensor_tensor(
            klo_i, key, ci[:NP, 3:4].to_broadcast([NP, 128]), Alu.bitwise_and
        )
        khi = kpool.tile([NP, 128], f32)
        nc.vector.tensor_copy(khi, khi_i)
        klo = kpool.tile([NP, 128], f32)
        nc.vector.tensor_copy(klo, klo_i)

        def count_descent(vals, mask, base, nbits, lo_init, label):
            """fp32 bitwise descent: returns max{v : base + #(mask & vals<=v) < kp}
            as a [NP,1] fp32 tile (mask/base optional)."""
            lo_t = kpool.tile([NP, 1], f32, name=f"lo{label}")
            nc.vector.memset(lo_t, float(lo_init))
            for b in range(nbits - 1, -1, -1):
                cand = kpool.tile([NP, 1], f32, tag="cand", name=f"cand{label}_{b}")
                nc.vector.tensor_scalar(cand, lo_t, float(2 ** b), None, Alu.add)
                lef = kpool.tile([NP, 128], f32, tag="lef", name=f"lef{label}_{b}")
                if mask is None:
                    nc.vector.tensor_scalar(lef, vals, cand, None, Alu.is_le)
                else:
                    nc.vector.scalar_tensor_tensor(
                        lef, vals, cand, mask, Alu.is_le, Alu.mult
                    )
                rs = kpool.tile([NP, 1], f32, tag="rs", name=f"rs{label}_{b}")
                nc.vector.tensor_reduce(rs, lef, AxX, Alu.add)
                cs = psK.tile([NP, 1], f32, tag="cs", name=f"cs{label}_{b}")
                nc.tensor.matmul(cs, hmat[:NP, :NP], rs, start=True, stop=True)
                cond = kpool.tile([NP, 1], f32, tag="cond", name=f"cond{label}_{b}")
                if base is None:
                    nc.vector.tensor_scalar(cond, cs, float(kp), None, Alu.is_lt)
                else:
                    nc.vector.tensor_scalar(cond, cs, base, float(kp), Alu.add, Alu.is_lt)
                step = kpool.tile([NP, 1], f32, tag="step", name=f"step{label}_{b}")
                nc.vector.tensor_scalar(step, cond, float(2 ** b), None, Alu.mult)
                nc.vector.tensor_tensor(lo_t, lo_t, step, Alu.add)
            res = kpool.tile([NP, 1], f32, name=f"thr{label}")
            nc.vector.tensor_scalar(res, lo_t, 1.0, None, Alu.add)
            return res

        # level 1: hi-part threshold H* (khi in [-2^23, 2^23))
        hstar = count_descent(khi, None, None, 24, -(2 ** 23) - 1, "H")
        lessH = kpool.tile([NP, 128], f32)
        nc.vector.tensor_scalar(lessH, khi, hstar, None, Alu.is_lt)
        eqH = kpool.tile([NP, 128], f32)
        nc.vector.tensor_scalar(eqH, khi, hstar, None, Alu.is_equal)
        rbh = kpool.tile([NP, 1], f32)
        nc.vector.tensor_reduce(rbh, lessH, AxX, Alu.add)
        cbp = psK.tile([NP, 1], f32, tag="cs", name="cbp")
        nc.tensor.matmul(cbp, hmat[:NP, :NP], rbh, start=True, stop=True)
        cbase = kpool.tile([NP, 1], f32)
        nc.vector.tensor_copy(cbase, cbp)
        # level 2: lo-part threshold among khi == H* (klo in [0, 256))
        lstar = count_descent(klo, eqH, cbase, 8, -1.0, "L")

        less = kpool.tile([NP, 128], f32)
        nc.vector.scalar_tensor_tensor(less, klo, lstar, eqH, Alu.is_lt, Alu.mult)
        nc.vector.tensor_tensor(less, less, lessH, Alu.add)
        eqf = kpool.tile([NP, 128], f32)
        nc.vector.scalar_tensor_tensor(eqf, klo, lstar, eqH, Alu.is_equal, Alu.mult)

        rl = kpool.tile([NP, 1], f32)
        nc.vector.tensor_reduce(rl, less, AxX, Alu.add)
        cls = psK.tile([NP, 1], f32, tag="cs", name="cls")
        nc.tensor.matmul(cls, hmat[:NP, :NP], rl, start=True, stop=True)
        clessf = kpool.tile([NP, 1], f32)
        nc.vector.tensor_copy(clessf, cls)

        re_ = kpool.tile([NP, 1], f32)
        nc.vector.tensor_reduce(re_, eqf, AxX, Alu.add)
        eqo = psK.tile([NP, 1], f32, tag="cs", name="eqo")
        nc.tensor.matmul(eqo, texcl[:NP, :NP], re_, start=True, stop=True)
        eqof = kpool.tile([NP, 1], f32)
        nc.vector.tensor_copy(eqof, eqo)

        # within-partition exclusive prefix of eq along f (log-shift adds)
        pa = kpool.tile([NP, 192], f32)
        pb = kpool.tile([NP, 192], f32)
        nc.vector.memset(pa[:, 0:64], 0.0)
        nc.vector.memset(pb[:, 0:64], 0.0)
        nc.vector.tensor_copy(pa[:, 64:192], eqf)
        cur, nxt = pa, pb
        for sh in (1, 2, 4, 8, 16, 32, 64):
            nc.vector.tensor_tensor(
                nxt[:, 64:192], cur[:, 64:192], cur[:, 64 - sh : 192 - sh], Alu.add
            )
            cur, nxt = nxt, cur
        excl = kpool.tile([NP, 128], f32)
        nc.vector.tensor_tensor(excl, cur[:, 64:192], eqf, Alu.subtract)
        exoff = kpool.tile([NP, 128], f32)
        nc.vector.tensor_scalar(exoff, excl, eqof, None, Alu.add)
        # keep the tied element iff (#ties before it) + c_less < kp
        kec = kpool.tile([NP, 128], f32)
        nc.vector.tensor_scalar(kec, exoff, clessf, float(kp), Alu.add, Alu.is_lt)
        keq = kpool.tile([NP, 128], f32)
        nc.vector.tensor_tensor(keq, kec, eqf, Alu.mult)
        keep = kpool.tile([NP, 128], f32)
        nc.vector.tensor_tensor(keep, less, keq, Alu.add)
        nc.vector.tensor_tensor(keep, keep, fone[:NP, :], Alu.add)
        yh8 = kpool.tile([NP, 128], u8)
        nc.vector.tensor_copy(yh8, keep)

        ysf = kpool.tile([NP, 128], f32)
        nc.scalar.activation(ysf, z2, Act.Sigmoid)

        nc.sync.dma_start(out=ys_d[:].rearrange("(p f) -> p f", f=128), in_=ysf)
        nc.sync.dma_start(out=yh_d[:].rearrange("(p f) -> p f", f=128), in_=yh8)

    nc.compile()
    return nc


def _host_consts(S, D, W3):
    """Host-precomputed constant tensors shared by all cores."""
    SB = S // 128
    KC = D // 128
    ident = np.eye(128, dtype=np.float32)
    blk = np.arange(128) // SB
    hmat = (blk[:, None] == blk[None, :]).astype(np.float32)
    iidx = np.arange(128)
    texcl = ((blk[:, None] == blk[None, :]) & (iidx[:, None] < iidx[None, :])).astype(
        np.float32
    )
    first = (np.arange(128) % SB == 0)[:, None] & (np.arange(128) == 0)[None, :]
    fbig = np.where(first, np.float32(3.0e38), np.float32(-3.0e38)).astype(np.float32)
    fone = first.astype(np.float32)
    vals = [31, 0x7FFFFFFF, 8, 0xFF]
    ci = np.tile(np.array(vals, dtype=np.int64).astype(np.int32), (128, 1))
    w3e = np.zeros((128, KC, 4, 4), dtype=np.float32)
    w3r = np.asarray(W3, dtype=np.float32).reshape(KC, 128).T  # [p, kc]
    for s in range(4):
        w3e[:, :, s, s] = w3r
    w3e = np.ascontiguousarray(w3e.reshape(128, KC * 16))
    return ident, hmat, texcl, fbig, fone, ci, w3e


_NC_CACHE = {}

# test-harness hooks: set TRACE=True before calling kernel() to profile; the
# BassKernelResults of the last run (with exec_time_ns when traced) lands in
# LAST_RESULTS.
TRACE = False
TRACE_TMPDIR = None
LAST_RESULTS = None
# fp16 3-term split for the two big matmul layers (~4/3 PE speedup, error
# ~2^-22 relative — validated to produce an identical y_hard mask)
USE_FP16 = True


_LDW_OPT_PATCHED = False


def _enable_ldw_opt():
    """walrus's redundant-LDWEIGHTS elimination is off by default in
    bass_utils; it is a ~16% win for this all-fp32 kernel (verified
    bit-identical outputs).  Patch the flag in; kernel() falls back to the
    stock flags if compilation fails."""
    global _LDW_OPT_PATCHED
    import concourse.bass_utils as bu

    if _LDW_OPT_PATCHED:
        return
    orig = bu.run_command

    def patched(argv, **kwargs):
        argv = [a.replace("--enable-ldw-opt=false", "--enable-ldw-opt=true")
                if isinstance(a, str) else a for a in argv]
        return orig(argv, **kwargs)

    bu.run_command = patched
    bu._ldw_opt_orig_run_command = orig
    _LDW_OPT_PATCHED = True


def _disable_ldw_opt():
    global _LDW_OPT_PATCHED
    import concourse.bass_utils as bu

    if _LDW_OPT_PATCHED and hasattr(bu, "_ldw_opt_orig_run_command"):
        bu.run_command = bu._ldw_opt_orig_run_command
        _LDW_OPT_PATCHED = False


def kernel(inputs, W1, b1, W2, b2, W3, b3, k):
    from concourse.bass_utils import run_bass_kernel_spmd

    x = np.asarray(inputs, dtype=np.float32)
    W1 = np.asarray(W1, dtype=np.float32)
    W2 = np.asarray(W2, dtype=np.float32)
    W3 = np.asarray(W3, dtype=np.float32)
    b1 = np.asarray(b1, dtype=np.float32)
    b2 = np.asarray(b2, dtype=np.float32)
    b3 = np.asarray(b3, dtype=np.float32)
    k = int(k)

    Bp, Sp, Dp = x.shape
    b_local = Bp // N_CORES
    T = b_local * Sp
    has_b1 = bool(np.any(b1 != 0))
    has_b2 = bool(np.any(b2 != 0))
    b3val = float(b3.reshape(-1)[0])

    fp16 = USE_FP16
    ckey = (b_local, Sp, Dp, k, has_b1, has_b2, b3val, fp16)
    if ckey not in _NC_CACHE:
        _NC_CACHE[ckey] = _build_nc(b_local, Sp, Dp, k, has_b1, has_b2, b3val,
                                    fp16=fp16)
    nc = _NC_CACHE[ckey]

    ident, hmat, texcl, fbig, fone, ci, w3e = _host_consts(Sp, Dp, W3)

    shared = {
        "w3e": w3e, "ident": ident, "hmat": hmat,
        "texcl": texcl, "fbig": fbig, "fone": fone, "ci": ci,
    }
    if fp16:
        shared["ident"] = ident.astype(np.float16)
        for nm, w in (("w1", W1), ("w2", W2)):
            wt = np.ascontiguousarray(w.T).astype(np.float32)
            wh = wt.astype(np.float16)
            wl = ((wt - wh.astype(np.float32)) * 2048.0).astype(np.float16)
            shared[nm + "h"] = wh
            shared[nm + "l"] = wl
    else:
        shared["w1t"] = np.ascontiguousarray(W1.T)
        shared["w2t"] = np.ascontiguousarray(W2.T)
    if has_b1:
        shared["b1r"] = np.ascontiguousarray(b1.reshape(1, Dp))
    if has_b2:
        shared["b2r"] = np.ascontiguousarray(b2.reshape(1, Dp))
    if has_b1 or has_b2:
        shared["ones"] = np.ones((1, 512), dtype=np.float32)

    in_maps = []
    for c in range(N_CORES):
        shard = np.ascontiguousarray(
            x[c * b_local : (c + 1) * b_local].reshape(T, Dp)
        )
        in_maps.append({"x": shard, **shared})

    global LAST_RESULTS
    kw = {}
    if TRACE and TRACE_TMPDIR:
        kw["tmpdir"] = TRACE_TMPDIR
    if fp16:
        _disable_ldw_opt()  # walrus ldw-opt miscompiles fp16 LDWEIGHTS
    else:
        _enable_ldw_opt()
    try:
        res = run_bass_kernel_spmd(
            nc, in_maps, core_ids=list(range(N_CORES)), trace=TRACE, **kw
        )
    except Exception:
        # fall back to stock walrus flags
        _disable_ldw_opt()
        res = run_bass_kernel_spmd(
            nc, in_maps, core_ids=list(range(N_CORES)), trace=TRACE, **kw
        )
    LAST_RESULTS = res
    y_soft = np.concatenate(
        [r["y_soft"].reshape(b_local, Sp) for r in res.results], axis=0
    )
    y_hard = np.concatenate(
        [r["y_hard"].reshape(b_local, Sp) for r in res.results], axis=0
    ).astype(bool)
    return (y_hard, y_soft)
